# revision 11
# baseline (speedup 1.0000x reference)
"""Trainium2 Bass kernel for nn_ClassDiagramGNN: 2-layer GAT on 50k nodes / 850k edges.

Strategy (8 NeuronCores, dst-sharded graph parallel):
  - Host: add self-loops, global LPT of nodes onto cores by degree, per-core
    LPT into 128-wide dst blocks, permute node ids so each block is
    contiguous. Per block, the 128 self-loop edges form a dedicated tile
    whose source rows are a contiguous slice of the local shard (direct DMA,
    no gather). Remaining edges are bucketed by (block, src-half) and packed
    into 128-edge tiles; trailing pad slots get idx=-1 so the gather skips
    them. The one-hot edge->dst scatter matrix S and its transpose ST are
    precomputed per tile in bf16 and streamed in by DMA.
  - Phase A: resident xT; per block h1 = x @ W1 plus attention scalars
    (folded into the matmul), packed as 640-col bf16 rows -> AllGather.
    Per-node adst scalars also go to a compact local table.
  - Edge pass (both layers), software-pipelined per block: prefetch DMAs +
    gathers for block b+1 while computing block b. Attention: ae = ST @ adst
    per tile accumulated into one PSUM strip, then batched add/leaky/exp over
    [128, T*NH]; per-tile w = p * h (split across DVE and ACT), aggregate via
    bf16 S^T @ w matmuls in PSUM, denominator S^T @ p, normalize, ELU,
    h2 @ W2 -> 256-col bf16 rows -> AllGather -> layer-2 pass -> fp32 out.
"""
import sys

for _p in ("/opt/trn_rl_repo",):
    if _p not in sys.path:
        sys.path.append(_p)

import heapq
import numpy as np
import ml_dtypes

import concourse.bass as bass
import concourse.bacc as bacc
import concourse.tile as tile
from concourse import mybir
from concourse import bass_utils

F32 = mybir.dt.float32
BF16 = mybir.dt.bfloat16
FP8 = mybir.dt.float8e4
I16 = mybir.dt.int16
AF = mybir.ActivationFunctionType
OP = mybir.AluOpType
NPBF = ml_dtypes.bfloat16
NPF8 = ml_dtypes.float8_e4m3

# problem constants (hardcoded per contract)
N, F_IN, HID, H1, E = 50000, 512, 128, 4, 800000
NEG = 0.2
C = 8                 # cores
NS = N // C           # 6250 nodes per shard
NBLK = (NS + 127) // 128   # 49 blocks per core
CAPS = [128] * (NBLK - 1) + [NS - 128 * (NBLK - 1)]  # 48x128 + 106
NBLK2 = (NBLK + 1) // 2    # blocks in the first gather half
NS2 = min(NBLK2 * 128, NS)  # within-shard gather-table half split (block-aligned)
ROW1 = 640            # layer-1 bf16 row: 512 feat + 4 asrc + pad (1280B)
ROW2 = 256            # layer-2 bf16 row: 128 feat + 1 asrc + pad (512B)
EPS = 1e-16

_cache = {}


def _reconfigure(n, e):
    """Testing hook: shrink the graph (keeps F_IN/HID/H1 fixed)."""
    global N, E, NS, NBLK, CAPS, NBLK2, NS2
    N, E = n, e
    NS = N // C
    NBLK = (NS + 127) // 128
    CAPS = [128] * (NBLK - 1) + [NS - 128 * (NBLK - 1)]
    NBLK2 = (NBLK + 1) // 2
    NS2 = min(NBLK2 * 128, NS)
    _cache.clear()


# --------------------------------------------------------------------------
# host-side preprocessing
# --------------------------------------------------------------------------

def _prepare(x, edge_index, W1, a_src1, a_dst1, b1, W2, a_src2, a_dst2, b2):
    # self-loops handled as dedicated per-block tiles; bucket only real edges
    src = edge_index[0].astype(np.int64)
    dst = edge_index[1].astype(np.int64)
    deg = np.bincount(dst, minlength=N) + 1  # +1 self loop

    # global LPT of nodes onto cores by degree (balances edge counts), then
    # per-core LPT into blocks
    perm_pos = np.empty(N, dtype=np.int64)    # orig id -> permuted global pos
    perm_order = np.empty(N, dtype=np.int64)  # permuted pos -> orig id
    order_g = np.argsort(-deg, kind="stable")
    cheap = [(0, 0, ci) for ci in range(C)]
    heapq.heapify(cheap)
    core_nodes = [[] for _ in range(C)]
    for nid in order_g:
        while True:
            load, used, ci = heapq.heappop(cheap)
            if used < NS:
                break
        core_nodes[ci].append(nid)
        heapq.heappush(cheap, (load + int(deg[nid]), used + 1, ci))
    for c in range(C):
        ids = np.array(core_nodes[c])
        d = deg[ids]
        order = np.argsort(-d, kind="stable")
        heap = [(0, 0, i) for i in range(NBLK)]
        heapq.heapify(heap)
        assign = [[] for _ in range(NBLK)]
        for li in order:
            while True:
                load, used, bi = heapq.heappop(heap)
                if used < CAPS[bi]:
                    break
            assign[bi].append(li)
            heapq.heappush(heap, (load + int(d[li]), used + 1, bi))
        pos = 0
        for bi in range(NBLK):
            for li in assign[bi]:
                g = c * NS + pos
                perm_pos[ids[li]] = g
                perm_order[g] = ids[li]
                pos += 1

    src_p = perm_pos[src]
    dst_p = perm_pos[dst]
    core = dst_p // NS
    blk = (dst_p % NS) // 128
    srcc = src_p // NS
    srcr = src_p % NS
    halfv = (srcr >= NS2).astype(np.int64)
    lorow = srcc * NS2 + srcr                 # row in lo chunk table
    hirow = srcc * (NS - NS2) + srcr - NS2    # row in hi chunk table
    key = (core * NBLK + blk) * 2 + halfv
    eorder = np.argsort(key, kind="stable")
    counts = np.bincount(key, minlength=C * NBLK * 2).reshape(C, NBLK, 2)

    # cross-core uniform tile counts per (block, half); +1 self tile per block
    T_lo = -(-counts[:, :, 0].max(axis=0) // 128)  # ceil
    T_hi = -(-counts[:, :, 1].max(axis=0) // 128)
    T_all = 1 + T_lo + T_hi                        # self tile first
    TT = int(T_all.sum())
    Tmax = int(T_all.max())
    toff = np.zeros(NBLK, np.int64)
    toff[1:] = np.cumsum(T_all)[:-1]

    src_sorted = np.where(halfv == 0, lorow, hirow)[eorder]
    dloc_sorted = (dst_p[eorder] % NS) % 128
    starts = np.zeros(C * NBLK * 2 + 1, np.int64)
    starts[1:] = np.cumsum(counts.reshape(-1))

    idx_all = np.zeros((C, TT * 128), np.int16)           # pad -> row 0
    dc_all = np.full((C, TT * 128), 999, np.int64)        # pad -> no dst
    nreal = np.zeros((C, NBLK, 2), np.int64)
    for c in range(C):
        for b in range(NBLK):
            # self tile: dst-local row i <- node base+i (contiguous source)
            bs = CAPS[b]
            slot0 = toff[b] * 128
            dc_all[c, slot0:slot0 + bs] = np.arange(bs)
            for h in range(2):
                k = (c * NBLK + b) * 2 + h
                s0, s1 = starts[k], starts[k + 1]
                n = s1 - s0
                nreal[c, b, h] = n
                if n == 0:
                    continue
                slot0 = (toff[b] + 1 + (T_lo[b] if h else 0)) * 128
                seg = src_sorted[s0:s1]
                idx_all[c, slot0:slot0 + n] = seg.astype(np.int16)
                dc_all[c, slot0:slot0 + n] = dloc_sorted[s0:s1]

    # full-range gathers (negative-index skipping crashes the gather ucode,
    # so pad slots gather row 0 instead)
    g_lo = T_lo * 128
    g_hi = T_hi * 128

    # weights: fold per-head attention projections into the linear transforms
    W1_64 = np.asarray(W1, np.float64)
    Dsrc1 = np.zeros((H1 * HID, H1), np.float64)
    Ddst1 = np.zeros((H1 * HID, H1), np.float64)
    a_src1_64 = np.asarray(a_src1, np.float64)
    a_dst1_64 = np.asarray(a_dst1, np.float64)
    for h in range(H1):
        Dsrc1[h * HID:(h + 1) * HID, h] = a_src1_64[h]
        Ddst1[h * HID:(h + 1) * HID, h] = a_dst1_64[h]
    rhs1 = np.concatenate(
        [np.asarray(W1, np.float32),
         (W1_64 @ Dsrc1).astype(np.float32),
         (W1_64 @ Ddst1).astype(np.float32)], axis=1).astype(NPBF)  # [512, 520]
    W2_64 = np.asarray(W2, np.float64)
    rhs2 = np.concatenate(
        [np.asarray(W2, np.float32),
         (W2_64 @ np.asarray(a_src2, np.float64)[0][:, None]).astype(np.float32),
         (W2_64 @ np.asarray(a_dst2, np.float64)[0][:, None]).astype(np.float32)],
        axis=1).astype(NPBF)                                        # [512, 130]

    ident = np.eye(128, dtype=NPBF)
    b1r = np.tile(np.asarray(b1, np.float32)[None, :], (128, 1))
    b2r = np.tile(np.asarray(b2, np.float32)[None, :], (128, 1))

    slots = np.arange(TT * 128)
    tloc = slots // 128
    posi = slots % 128

    xnp = np.asarray(x, np.float32)
    in_maps = []
    for c in range(C):
        rows = perm_order[c * NS:(c + 1) * NS]
        xT = np.ascontiguousarray(xnp[rows].T).astype(NPBF)     # [512, 6250]
        idx_w = np.tile(idx_all[c].reshape(-1, 16).T, (8, 1))   # [128, TT*8]
        dcv = dc_all[c]
        valid = dcv < 128
        dv = dcv[valid]
        tv = tloc[valid]
        pv = posi[valid]
        S_all = np.zeros((128, TT * 128), NPF8)
        S_all[pv, tv * 128 + dv] = 1
        ST_all = np.zeros((128, TT * 128), NPF8)
        ST_all[dv, tv * 128 + pv] = 1
        in_maps.append({
            "xT": xT, "rhs1": rhs1, "rhs2": rhs2,
            "b1r": b1r, "b2r": b2r, "ident": ident,
            "idx": np.ascontiguousarray(idx_w),
            "S": S_all, "ST": ST_all,
        })

    meta = {
        "T_lo": [int(v) for v in T_lo],
        "T_hi": [int(v) for v in T_hi],
        "toff": [int(v) for v in toff],
        "g_lo": [int(v) for v in g_lo],
        "g_hi": [int(v) for v in g_hi],
        "TT": TT,
        "Tmax": Tmax,
    }
    return in_maps, meta, perm_order


# --------------------------------------------------------------------------
# device program
# --------------------------------------------------------------------------

def _edge_layer(nc, tc, meta, lay, pools, ad_my, hb_pair, hflo, hfhi, out_writer,
                post_block=None):
    """Software-pipelined per-block edge pass shared by both GAT layers.

    lay=1: ROW=640, 4 heads, feat cols 0:512, asrc 512:516
    lay=2: ROW=256, 1 head, feat cols 0:128, asrc 128:129
    """
    sbm, sbg, sbs, sbw, psb, psa = pools
    ROW = ROW1 if lay == 1 else ROW2
    NH = H1 if lay == 1 else 1
    FEAT = NH * HID
    ACOL = FEAT
    idx_d, S_d, ST_d = meta["idx_ap"], meta["S_ap"], meta["ST_ap"]
    Tmax = meta["Tmax"]
    _l = hflo[:, :, :]
    lo_ap = bass.AP(_l.tensor, _l.offset, [[ROW, C * NS2], [1, ROW]])
    _h = hfhi[:, :, :]
    hi_ap = bass.AP(_h.tensor, _h.offset, [[ROW, C * (NS - NS2)], [1, ROW]])

    # one-time zero of the gather-pool buffers so skipped pad slots never
    # expose uninitialized SBUF (NaN/Inf) to the attention math
    for _ in range(sbg.bufs):
        gz = sbg.tile([128, Tmax, ROW], BF16, tag="gat")
        nc.vector.memset(gz[:, :, :], 0.0)

    def prefetch(b):
        bs = CAPS[b]
        base = b * 128
        T_lo, T_hi = meta["T_lo"][b], meta["T_hi"][b]
        T = 1 + T_lo + T_hi
        boff = meta["toff"][b]
        g_lo, g_hi = meta["g_lo"][b], meta["g_hi"][b]

        idx_sb = sbm.tile([128, Tmax * 8], I16, tag="idx")
        nc.sync.dma_start(idx_sb[:, 0:T * 8], idx_d[:, boff * 8:(boff + T) * 8])
        S_sb = sbm.tile([128, Tmax * 128], FP8, tag="S")
        nc.sync.dma_start(S_sb[:, 0:T * 128], S_d[:, boff * 128:(boff + T) * 128])
        ST_sb = sbm.tile([128, Tmax * 128], FP8, tag="ST")
        nc.sync.dma_start(ST_sb[:, 0:T * 128], ST_d[:, boff * 128:(boff + T) * 128])
        adst_sb = sbm.tile([128, NH], BF16, tag="adst")
        if bs < 128:
            nc.vector.memset(adst_sb[:], 0.0)
        nc.sync.dma_start(adst_sb[:bs], ad_my[base:base + bs, :])

        gat = sbg.tile([128, Tmax, ROW], BF16, tag="gat")
        # self tile: contiguous local rows
        if base < NS2:
            nc.sync.dma_start(gat[:bs, 0, :], hb_pair[0][base:base + bs, :])
        else:
            nc.sync.dma_start(gat[:bs, 0, :], hb_pair[1][base - NS2:base - NS2 + bs, :])
        if g_lo:
            nc.gpsimd.dma_gather(
                gat[:, 1:1 + g_lo // 128, :], lo_ap,
                idx_sb[:, 8:(1 + g_lo // 128) * 8],
                g_lo, g_lo, ROW, elem_step=ROW, single_packet=False)
        if g_hi:
            t0 = 1 + T_lo
            nc.gpsimd.dma_gather(
                gat[:, t0:t0 + g_hi // 128, :], hi_ap,
                idx_sb[:, t0 * 8:(t0 + g_hi // 128) * 8],
                g_hi, g_hi, ROW, elem_step=ROW, single_packet=False)
        return (b, bs, base, T, idx_sb, S_sb, ST_sb, adst_sb, gat)

    def compute(st):
        b, bs, base, T, idx_sb, S_sb, ST_sb, adst_sb, gat = st
        ae = psa.tile([128, Tmax * NH + NH], F32, tag="ae")
        for t in range(T):
            nc.tensor.matmul(ae[:, t * NH:(t + 1) * NH],
                             ST_sb[:, t * 128:(t + 1) * 128], adst_sb[:],
                             start=True, stop=True)
        ep = sbs.tile([128, Tmax * NH], BF16, tag="ep")
        nc.vector.tensor_tensor(ep[:, 0:T * NH], ae[:, 0:T * NH],
                                gat[:, 0:T, ACOL:ACOL + NH], OP.add)
        lr = sbs.tile([128, Tmax * NH], BF16, tag="lr")
        nc.vector.scalar_tensor_tensor(lr[:, 0:T * NH], ep[:, 0:T * NH],
                                       NEG, ep[:, 0:T * NH], OP.mult, OP.max)
        p = sbs.tile([128, Tmax * NH], F32, tag="p")
        nc.scalar.activation(p[:, 0:T * NH], lr[:, 0:T * NH], AF.Exp)
        p16 = sbs.tile([128, Tmax * NH], BF16, tag="p16")
        nc.scalar.activation(p16[:, 0:T * NH], p[:, 0:T * NH], AF.Copy)

        oacc = psb.tile([128, FEAT], F32, tag="oacc")
        dacc = ae[:, Tmax * NH:Tmax * NH + NH]
        for t in range(T):
            w = sbw.tile([128, FEAT], BF16, tag="w")
            if lay == 1:
                nc.scalar.activation(w[:, 0:HID], gat[:, t, 0:HID],
                                     AF.Copy, scale=p[:, t * NH:t * NH + 1])
                for h in range(1, 4):
                    nc.vector.tensor_scalar_mul(
                        w[:, h * HID:(h + 1) * HID], gat[:, t, h * HID:(h + 1) * HID],
                        p[:, t * NH + h:t * NH + h + 1])
            else:
                if t % 2 == 0:
                    nc.scalar.activation(w[:], gat[:, t, 0:FEAT], AF.Copy,
                                         scale=p[:, t:t + 1])
                else:
                    nc.vector.tensor_scalar_mul(w[:], gat[:, t, 0:FEAT],
                                                p[:, t:t + 1])
            nc.tensor.matmul(oacc[:], S_sb[:, t * 128:(t + 1) * 128], w[:],
                             start=(t == 0), stop=(t == T - 1))
            nc.tensor.matmul(dacc, S_sb[:, t * 128:(t + 1) * 128],
                             p16[:, t * NH:(t + 1) * NH],
                             start=(t == 0), stop=(t == T - 1))
        out_writer(b, base, bs, oacc, dacc)

    st = prefetch(0)
    for b in range(NBLK):
        nxt = prefetch(b + 1) if b + 1 < NBLK else None
        compute(st)
        if post_block is not None:
            post_block(b)
        st = nxt


def _build(meta):
    nc = bacc.Bacc("TRN2", target_bir_lowering=False, debug=False, num_devices=C)
    TT = meta["TT"]

    xT_d = nc.dram_tensor("xT", [F_IN, NS], BF16, kind="ExternalInput").ap()
    rhs1_d = nc.dram_tensor("rhs1", [F_IN, 520], BF16, kind="ExternalInput").ap()
    rhs2_d = nc.dram_tensor("rhs2", [F_IN, 130], BF16, kind="ExternalInput").ap()
    b1r_d = nc.dram_tensor("b1r", [128, 512], F32, kind="ExternalInput").ap()
    b2r_d = nc.dram_tensor("b2r", [128, 128], F32, kind="ExternalInput").ap()
    ident_d = nc.dram_tensor("ident", [128, 128], BF16, kind="ExternalInput").ap()
    idx_d = nc.dram_tensor("idx", [128, TT * 8], I16, kind="ExternalInput").ap()
    S_d = nc.dram_tensor("S", [128, TT * 128], FP8, kind="ExternalInput").ap()
    ST_d = nc.dram_tensor("ST", [128, TT * 128], FP8, kind="ExternalInput").ap()
    out_d = nc.dram_tensor("out", [NS, HID], F32, kind="ExternalOutput").ap()

    meta = dict(meta)
    meta["idx_ap"], meta["S_ap"], meta["ST_ap"] = idx_d, S_d, ST_d

    with tile.TileContext(nc, num_cores=C) as tc:
        with tc.tile_pool(name="dram", bufs=1, space="DRAM") as dram:
            hb1a = dram.tile([NS2, ROW1], BF16)
            hb1b = dram.tile([NS - NS2, ROW1], BF16)
            hf1a = dram.tile([C, NS2, ROW1], BF16, addr_space="Shared")
            hf1b = dram.tile([C, NS - NS2, ROW1], BF16, addr_space="Shared")
            ad1 = dram.tile([NS, H1], BF16)
            hb2a = dram.tile([NS2, ROW2], BF16)
            hb2b = dram.tile([NS - NS2, ROW2], BF16)
            hf2a = dram.tile([C, NS2, ROW2], BF16, addr_space="Shared")
            hf2b = dram.tile([C, NS - NS2, ROW2], BF16, addr_space="Shared")
            ad2 = dram.tile([NS, 1], BF16)

            bmid = NBLK2 - 1  # emit first-half AG after this block

            def hb_slice(ha_pair, base, bs):
                if base < NS2:
                    return ha_pair[0][base:base + bs, :]
                return ha_pair[1][base - NS2:base - NS2 + bs, :]

            def ag_half(hb, hf):
                nc.gpsimd.collective_compute(
                    "AllGather", OP.bypass, replica_groups=[list(range(C))],
                    ins=[hb[:].opt()], outs=[hf[:, :, :].opt()])

            # ---------------- phase A: h1 shard + attn scalars ----------------
            with (
                tc.tile_pool(name="a_c", bufs=1) as sbc,
                tc.tile_pool(name="a_w", bufs=3) as sbw,
                tc.tile_pool(name="a_p", bufs=2, space="PSUM") as psp,
            ):
                rhs1_sb = []
                xt_sb = []
                for k in range(4):
                    rt = sbc.tile([128, 520], BF16, name=f"rhs1sb{k}")
                    nc.sync.dma_start(rt[:], rhs1_d[k * 128:(k + 1) * 128, :])
                    rhs1_sb.append(rt)
                    xt = sbc.tile([128, NS], BF16, name=f"xtsb{k}")
                    nc.sync.dma_start(xt[:], xT_d[k * 128:(k + 1) * 128, :])
                    xt_sb.append(xt)
                for b in range(NBLK):
                    bs = CAPS[b]
                    base = b * 128
                    ph = psp.tile([128, 512], F32, tag="ph")
                    pa = psp.tile([128, 8], F32, tag="pa")
                    for k in range(4):
                        nc.tensor.matmul(ph[:bs, :], xt_sb[k][:, base:base + bs],
                                         rhs1_sb[k][:, 0:512],
                                         start=(k == 0), stop=(k == 3))
                        nc.tensor.matmul(pa[:bs, :], xt_sb[k][:, base:base + bs],
                                         rhs1_sb[k][:, 512:520],
                                         start=(k == 0), stop=(k == 3))
                    ha = sbw.tile([128, ROW1], BF16, tag="ha")
                    nc.scalar.activation(ha[:bs, 0:512], ph[:bs, :], AF.Copy)
                    nc.scalar.activation(ha[:bs, 512:516], pa[:bs, 0:4], AF.Copy)
                    nc.vector.memset(ha[:bs, 516:ROW1], 0.0)
                    nc.sync.dma_start(hb_slice((hb1a, hb1b), base, bs), ha[:bs, :])
                    adsb = sbw.tile([128, H1], BF16, tag="adsb")
                    nc.scalar.activation(adsb[:bs, :], pa[:bs, 4:8], AF.Copy)
                    nc.sync.dma_start(ad1[base:base + bs, :], adsb[:bs, :])
                    if b == bmid:
                        ag_half(hb1a, hf1a)
                    elif b == NBLK - 1:
                        ag_half(hb1b, hf1b)

            # ---------------- phase B: layer-1 edge pass + h2@W2 ----------------
            with (
                tc.tile_pool(name="b_c", bufs=1) as sbc,
                tc.tile_pool(name="b_m", bufs=2) as sbm,
                tc.tile_pool(name="b_g", bufs=2) as sbg,
                tc.tile_pool(name="b_s", bufs=2) as sbs,
                tc.tile_pool(name="b_w", bufs=4) as sbw,
                tc.tile_pool(name="b_w2", bufs=2) as sbw2,
                tc.tile_pool(name="b_pb", bufs=2, space="PSUM") as psb,
                tc.tile_pool(name="b_pa", bufs=2, space="PSUM") as psa,
                tc.tile_pool(name="b_ph", bufs=1, space="PSUM") as psh,
                tc.tile_pool(name="b_pt", bufs=2, space="PSUM") as pst,
            ):
                b1r_sb = sbc.tile([128, 512], F32, name="b1rsb")
                nc.sync.dma_start(b1r_sb[:], b1r_d)
                ident_sb = sbc.tile([128, 128], BF16, name="identsb")
                nc.sync.dma_start(ident_sb[:], ident_d)
                rhs2_sb = []
                for k in range(4):
                    rt = sbc.tile([128, 130], BF16, name=f"rhs2sb{k}")
                    nc.sync.dma_start(rt[:], rhs2_d[k * 128:(k + 1) * 128, :])
                    rhs2_sb.append(rt)

                def writer_b(b, base, bs, oacc, dacc):
                    den = sbw2.tile([128, 4], F32, tag="den")
                    nc.vector.tensor_scalar_add(den[:], dacc, EPS)
                    rec = sbw2.tile([128, 4], F32, tag="rec")
                    nc.vector.reciprocal(rec[:], den[:])
                    h2b = sbw2.tile([128, 512], BF16, tag="h2b")
                    for h in range(4):
                        nc.vector.scalar_tensor_tensor(
                            h2b[:, h * HID:(h + 1) * HID], oacc[:, h * HID:(h + 1) * HID],
                            rec[:, h:h + 1], b1r_sb[:, h * HID:(h + 1) * HID],
                            OP.mult, OP.add)
                    rl = sbw2.tile([128, 512], BF16, tag="rl")
                    nc.scalar.activation(rl[:], h2b[:], AF.Relu)
                    mn = sbw2.tile([128, 512], BF16, tag="mn")
                    nc.vector.tensor_scalar_min(mn[:], h2b[:], 0.0)
                    em = sbw2.tile([128, 512], BF16, tag="em")
                    nc.scalar.activation(em[:], mn[:], AF.Exp)
                    h2f = sbw2.tile([128, 512], BF16, tag="h2f")
                    nc.vector.scalar_tensor_tensor(h2f[:], em[:], -1.0, rl[:], OP.add, OP.add)
                    hh = psh.tile([128, 130], F32, tag="hh")
                    for k in range(4):
                        tp = pst.tile([128, 128], BF16, tag="tp")
                        nc.tensor.transpose(tp[:], h2f[:, k * 128:(k + 1) * 128], ident_sb[:])
                        h2T = sbw2.tile([128, 128], BF16, tag="h2T")
                        nc.vector.tensor_copy(h2T[:], tp[:])
                        nc.tensor.matmul(hh[:], h2T[:], rhs2_sb[k][:], start=(k == 0), stop=(k == 3))
                    ha2 = sbw2.tile([128, ROW2], BF16, tag="ha2")
                    nc.scalar.activation(ha2[:bs, 0:129], hh[:bs, 0:129], AF.Copy)
                    nc.vector.memset(ha2[:bs, 129:ROW2], 0.0)
                    nc.sync.dma_start(hb_slice((hb2a, hb2b), base, bs), ha2[:bs, :])
                    adsb2 = sbw2.tile([128, 1], BF16, tag="adsb2")
                    nc.scalar.activation(adsb2[:bs, :], hh[:bs, 129:130], AF.Copy)
                    nc.sync.dma_start(ad2[base:base + bs, :], adsb2[:bs, :])

                def post_b(b):
                    if b == bmid:
                        ag_half(hb2a, hf2a)
                    elif b == NBLK - 1:
                        ag_half(hb2b, hf2b)

                _edge_layer(nc, tc, meta, 1, (sbm, sbg, sbs, sbw, psb, psa),
                            ad1, (hb1a, hb1b), hf1a, hf1b, writer_b, post_block=post_b)

            # ---------------- phase D: layer-2 edge pass ----------------
            with (
                tc.tile_pool(name="d_c", bufs=1) as sbc,
                tc.tile_pool(name="d_m", bufs=2) as sbm,
                tc.tile_pool(name="d_g", bufs=2) as sbg,
                tc.tile_pool(name="d_s", bufs=2) as sbs,
                tc.tile_pool(name="d_w", bufs=4) as sbw,
                tc.tile_pool(name="d_w2", bufs=2) as sbw2,
                tc.tile_pool(name="d_pb", bufs=2, space="PSUM") as psb,
                tc.tile_pool(name="d_pa", bufs=2, space="PSUM") as psa,
            ):
                b2r_sb = sbc.tile([128, 128], F32, name="b2rsb")
                nc.sync.dma_start(b2r_sb[:], b2r_d)

                def writer_d(b, base, bs, oacc, dacc):
                    den = sbw2.tile([128, 1], F32, tag="den")
                    nc.vector.tensor_scalar_add(den[:], dacc, EPS)
                    rec = sbw2.tile([128, 1], F32, tag="rec")
                    nc.vector.reciprocal(rec[:], den[:])
                    ofb = sbw2.tile([128, 128], F32, tag="ofb")
                    nc.vector.scalar_tensor_tensor(ofb[:], oacc[:], rec[:, 0:1],
                                                   b2r_sb[:], OP.mult, OP.add)
                    nc.sync.dma_start(out_d[base:base + bs, :], ofb[:bs, :])

                _edge_layer(nc, tc, meta, 2, (sbm, sbg, sbs, sbw, psb, psa),
                            ad2, (hb2a, hb2b), hf2a, hf2b, writer_d)

    nc.compile()
    return nc


# --------------------------------------------------------------------------
# entry point
# --------------------------------------------------------------------------

def kernel(x, edge_index, W1, a_src1, a_dst1, b1, W2, a_src2, a_dst2, b2,
           _trace=False):
    in_maps, meta, perm_order = _prepare(
        x, edge_index, W1, a_src1, a_dst1, b1, W2, a_src2, a_dst2, b2)

    import time as _time
    _t0 = _time.time()
    key = (meta["TT"], tuple(meta["T_lo"]), tuple(meta["T_hi"]),
           tuple(meta["g_lo"]), tuple(meta["g_hi"]))
    if key not in _cache:
        _cache.clear()
        _cache[key] = _build(meta)
    nc = _cache[key]
    print(f"[kernel] build done at {_time.time()-_t0:.1f}s", flush=True)

    kw = {}
    if _trace:
        kw = dict(trace=True)
    res = bass_utils.run_bass_kernel_spmd(nc, in_maps, core_ids=list(range(C)), **kw)

    out = np.empty((N, HID), np.float32)
    for c in range(C):
        out[perm_order[c * NS:(c + 1) * NS]] = res.results[c]["out"]
    kernel._last_result = res
    return out


# revision 12
# speedup vs baseline: 1.1797x; 1.1797x over previous
"""Trainium2 Bass kernel for nn_ClassDiagramGNN: 2-layer GAT on 50k nodes / 850k edges.

Strategy (8 NeuronCores, dst-sharded graph parallel):
  - Host: add self-loops, global LPT of nodes onto cores by degree, per-core
    LPT into 128-wide dst blocks, permute node ids so each block is
    contiguous. Per block, the 128 self-loop edges form a dedicated tile
    whose source rows are a contiguous slice of the local shard (direct DMA,
    no gather). Remaining edges are bucketed by (block, src-half) and packed
    into 128-edge tiles; trailing pad slots get idx=-1 so the gather skips
    them. The one-hot edge->dst scatter matrix S and its transpose ST are
    precomputed per tile in bf16 and streamed in by DMA.
  - Phase A: resident xT; per block h1 = x @ W1 plus attention scalars
    (folded into the matmul), packed as 640-col bf16 rows -> AllGather.
    Per-node adst scalars also go to a compact local table.
  - Edge pass (both layers), software-pipelined per block: prefetch DMAs +
    gathers for block b+1 while computing block b. Attention: ae = ST @ adst
    per tile accumulated into one PSUM strip, then batched add/leaky/exp over
    [128, T*NH]; per-tile w = p * h (split across DVE and ACT), aggregate via
    bf16 S^T @ w matmuls in PSUM, denominator S^T @ p, normalize, ELU,
    h2 @ W2 -> 256-col bf16 rows -> AllGather -> layer-2 pass -> fp32 out.
"""
import sys

for _p in ("/opt/trn_rl_repo",):
    if _p not in sys.path:
        sys.path.append(_p)

import heapq
import numpy as np
import ml_dtypes

import concourse.bass as bass
import concourse.bacc as bacc
import concourse.tile as tile
from concourse import mybir
from concourse import bass_utils

F32 = mybir.dt.float32
BF16 = mybir.dt.bfloat16
FP8 = mybir.dt.float8e4
I16 = mybir.dt.int16
AF = mybir.ActivationFunctionType
OP = mybir.AluOpType
NPBF = ml_dtypes.bfloat16
NPF8 = ml_dtypes.float8_e4m3

# problem constants (hardcoded per contract)
N, F_IN, HID, H1, E = 50000, 512, 128, 4, 800000
NEG = 0.2
C = 8                 # cores
NS = N // C           # 6250 nodes per shard
NBLK = (NS + 127) // 128   # 49 blocks per core
CAPS = [128] * (NBLK - 1) + [NS - 128 * (NBLK - 1)]  # 48x128 + 106
HALF = N // 2         # gather-table half split (int16 index reach)
ROW1 = 640            # layer-1 bf16 row: 512 feat + 4 asrc + pad (1280B)
ROW2 = 256            # layer-2 bf16 row: 128 feat + 1 asrc + pad (512B)
EPS = 1e-16

_cache = {}


def _reconfigure(n, e):
    """Testing hook: shrink the graph (keeps F_IN/HID/H1 fixed)."""
    global N, E, NS, NBLK, CAPS, HALF
    N, E = n, e
    NS = N // C
    NBLK = (NS + 127) // 128
    CAPS = [128] * (NBLK - 1) + [NS - 128 * (NBLK - 1)]
    HALF = N // 2
    _cache.clear()


# --------------------------------------------------------------------------
# host-side preprocessing
# --------------------------------------------------------------------------

def _prepare(x, edge_index, W1, a_src1, a_dst1, b1, W2, a_src2, a_dst2, b2):
    # self-loops handled as dedicated per-block tiles; bucket only real edges
    src = edge_index[0].astype(np.int64)
    dst = edge_index[1].astype(np.int64)
    deg = np.bincount(dst, minlength=N) + 1  # +1 self loop

    # global LPT of nodes onto cores by degree (balances edge counts), then
    # per-core LPT into blocks
    perm_pos = np.empty(N, dtype=np.int64)    # orig id -> permuted global pos
    perm_order = np.empty(N, dtype=np.int64)  # permuted pos -> orig id
    order_g = np.argsort(-deg, kind="stable")
    cheap = [(0, 0, ci) for ci in range(C)]
    heapq.heapify(cheap)
    core_nodes = [[] for _ in range(C)]
    for nid in order_g:
        while True:
            load, used, ci = heapq.heappop(cheap)
            if used < NS:
                break
        core_nodes[ci].append(nid)
        heapq.heappush(cheap, (load + int(deg[nid]), used + 1, ci))
    for c in range(C):
        ids = np.array(core_nodes[c])
        d = deg[ids]
        order = np.argsort(-d, kind="stable")
        heap = [(0, 0, i) for i in range(NBLK)]
        heapq.heapify(heap)
        assign = [[] for _ in range(NBLK)]
        for li in order:
            while True:
                load, used, bi = heapq.heappop(heap)
                if used < CAPS[bi]:
                    break
            assign[bi].append(li)
            heapq.heappush(heap, (load + int(d[li]), used + 1, bi))
        pos = 0
        for bi in range(NBLK):
            for li in assign[bi]:
                g = c * NS + pos
                perm_pos[ids[li]] = g
                perm_order[g] = ids[li]
                pos += 1

    src_p = perm_pos[src]
    dst_p = perm_pos[dst]
    core = dst_p // NS
    blk = (dst_p % NS) // 128
    halfv = (src_p >= HALF).astype(np.int64)
    key = (core * NBLK + blk) * 2 + halfv
    eorder = np.argsort(key, kind="stable")
    counts = np.bincount(key, minlength=C * NBLK * 2).reshape(C, NBLK, 2)

    # cross-core uniform tile counts per (block, half); +1 self tile per block
    T_lo = -(-counts[:, :, 0].max(axis=0) // 128)  # ceil
    T_hi = -(-counts[:, :, 1].max(axis=0) // 128)
    T_all = 1 + T_lo + T_hi                        # self tile first
    TT = int(T_all.sum())
    Tmax = int(T_all.max())
    toff = np.zeros(NBLK, np.int64)
    toff[1:] = np.cumsum(T_all)[:-1]

    src_sorted = src_p[eorder]
    dloc_sorted = (dst_p[eorder] % NS) % 128
    starts = np.zeros(C * NBLK * 2 + 1, np.int64)
    starts[1:] = np.cumsum(counts.reshape(-1))

    idx_all = np.zeros((C, TT * 128), np.int16)           # pad -> row 0
    dc_all = np.full((C, TT * 128), 999, np.int64)        # pad -> no dst
    nreal = np.zeros((C, NBLK, 2), np.int64)
    for c in range(C):
        for b in range(NBLK):
            # self tile: dst-local row i <- node base+i (contiguous source)
            bs = CAPS[b]
            slot0 = toff[b] * 128
            dc_all[c, slot0:slot0 + bs] = np.arange(bs)
            for h in range(2):
                k = (c * NBLK + b) * 2 + h
                s0, s1 = starts[k], starts[k + 1]
                n = s1 - s0
                nreal[c, b, h] = n
                if n == 0:
                    continue
                slot0 = (toff[b] + 1 + (T_lo[b] if h else 0)) * 128
                seg = src_sorted[s0:s1]
                if h:
                    seg = seg - HALF
                idx_all[c, slot0:slot0 + n] = seg.astype(np.int16)
                dc_all[c, slot0:slot0 + n] = dloc_sorted[s0:s1]

    # full-range gathers (negative-index skipping crashes the gather ucode,
    # so pad slots gather row 0 instead)
    g_lo = T_lo * 128
    g_hi = T_hi * 128

    # weights: fold per-head attention projections into the linear transforms
    W1_64 = np.asarray(W1, np.float64)
    Dsrc1 = np.zeros((H1 * HID, H1), np.float64)
    Ddst1 = np.zeros((H1 * HID, H1), np.float64)
    a_src1_64 = np.asarray(a_src1, np.float64)
    a_dst1_64 = np.asarray(a_dst1, np.float64)
    for h in range(H1):
        Dsrc1[h * HID:(h + 1) * HID, h] = a_src1_64[h]
        Ddst1[h * HID:(h + 1) * HID, h] = a_dst1_64[h]
    rhs1 = np.concatenate(
        [np.asarray(W1, np.float32),
         (W1_64 @ Dsrc1).astype(np.float32),
         (W1_64 @ Ddst1).astype(np.float32)], axis=1).astype(NPBF)  # [512, 520]
    W2_64 = np.asarray(W2, np.float64)
    rhs2 = np.concatenate(
        [np.asarray(W2, np.float32),
         (W2_64 @ np.asarray(a_src2, np.float64)[0][:, None]).astype(np.float32),
         (W2_64 @ np.asarray(a_dst2, np.float64)[0][:, None]).astype(np.float32)],
        axis=1).astype(NPBF)                                        # [512, 130]

    ident = np.eye(128, dtype=NPBF)
    b1r = np.tile(np.asarray(b1, np.float32)[None, :], (128, 1))
    b2r = np.tile(np.asarray(b2, np.float32)[None, :], (128, 1))

    slots = np.arange(TT * 128)
    tloc = slots // 128
    posi = slots % 128

    xnp = np.asarray(x, np.float32)
    in_maps = []
    for c in range(C):
        rows = perm_order[c * NS:(c + 1) * NS]
        xT = np.ascontiguousarray(xnp[rows].T).astype(NPBF)     # [512, 6250]
        idx_w = np.tile(idx_all[c].reshape(-1, 16).T, (8, 1))   # [128, TT*8]
        dcv = dc_all[c]
        valid = dcv < 128
        dv = dcv[valid]
        tv = tloc[valid]
        pv = posi[valid]
        S_all = np.zeros((128, TT * 128), NPF8)
        S_all[pv, tv * 128 + dv] = 1
        ST_all = np.zeros((128, TT * 128), NPF8)
        ST_all[dv, tv * 128 + pv] = 1
        in_maps.append({
            "xT": xT, "rhs1": rhs1, "rhs2": rhs2,
            "b1r": b1r, "b2r": b2r, "ident": ident,
            "idx": np.ascontiguousarray(idx_w),
            "S": S_all, "ST": ST_all,
        })

    meta = {
        "T_lo": [int(v) for v in T_lo],
        "T_hi": [int(v) for v in T_hi],
        "toff": [int(v) for v in toff],
        "g_lo": [int(v) for v in g_lo],
        "g_hi": [int(v) for v in g_hi],
        "TT": TT,
        "Tmax": Tmax,
    }
    return in_maps, meta, perm_order


# --------------------------------------------------------------------------
# device program
# --------------------------------------------------------------------------

def _edge_layer(nc, tc, meta, lay, pools, ad_my, hb_my, hfull, out_writer):
    """Software-pipelined per-block edge pass shared by both GAT layers.

    lay=1: ROW=640, 4 heads, feat cols 0:512, asrc 512:516
    lay=2: ROW=256, 1 head, feat cols 0:128, asrc 128:129
    """
    sbm, sbg, sbs, sbw, psb, psa = pools
    ROW = ROW1 if lay == 1 else ROW2
    NH = H1 if lay == 1 else 1
    FEAT = NH * HID
    ACOL = FEAT
    idx_d, S_d, ST_d = meta["idx_ap"], meta["S_ap"], meta["ST_ap"]
    Tmax = meta["Tmax"]

    # one-time zero of the gather-pool buffers so skipped pad slots never
    # expose uninitialized SBUF (NaN/Inf) to the attention math
    for _ in range(sbg.bufs):
        gz = sbg.tile([128, Tmax, ROW], BF16, tag="gat")
        nc.vector.memset(gz[:, :, :], 0.0)

    def prefetch(b):
        bs = CAPS[b]
        base = b * 128
        T_lo, T_hi = meta["T_lo"][b], meta["T_hi"][b]
        T = 1 + T_lo + T_hi
        boff = meta["toff"][b]
        g_lo, g_hi = meta["g_lo"][b], meta["g_hi"][b]

        idx_sb = sbm.tile([128, Tmax * 8], I16, tag="idx")
        nc.sync.dma_start(idx_sb[:, 0:T * 8], idx_d[:, boff * 8:(boff + T) * 8])
        S_sb = sbm.tile([128, Tmax * 128], FP8, tag="S")
        nc.sync.dma_start(S_sb[:, 0:T * 128], S_d[:, boff * 128:(boff + T) * 128])
        ST_sb = sbm.tile([128, Tmax * 128], FP8, tag="ST")
        nc.sync.dma_start(ST_sb[:, 0:T * 128], ST_d[:, boff * 128:(boff + T) * 128])
        adst_sb = sbm.tile([128, NH], BF16, tag="adst")
        if bs < 128:
            nc.vector.memset(adst_sb[:], 0.0)
        nc.sync.dma_start(adst_sb[:bs], ad_my[base:base + bs, :])

        gat = sbg.tile([128, Tmax, ROW], BF16, tag="gat")
        # self tile: contiguous local rows
        nc.sync.dma_start(gat[:bs, 0, :], hb_my[base:base + bs, :])
        if g_lo:
            nc.gpsimd.dma_gather(
                gat[:, 1:1 + g_lo // 128, :], hfull[0:HALF, :],
                idx_sb[:, 8:(1 + g_lo // 128) * 8],
                g_lo, g_lo, ROW, elem_step=ROW, single_packet=False)
        if g_hi:
            t0 = 1 + T_lo
            nc.gpsimd.dma_gather(
                gat[:, t0:t0 + g_hi // 128, :], hfull[HALF:N, :],
                idx_sb[:, t0 * 8:(t0 + g_hi // 128) * 8],
                g_hi, g_hi, ROW, elem_step=ROW, single_packet=False)
        return (b, bs, base, T, idx_sb, S_sb, ST_sb, adst_sb, gat)

    def compute(st):
        b, bs, base, T, idx_sb, S_sb, ST_sb, adst_sb, gat = st
        ae = psa.tile([128, Tmax * NH + NH], F32, tag="ae")
        for t in range(T):
            nc.tensor.matmul(ae[:, t * NH:(t + 1) * NH],
                             ST_sb[:, t * 128:(t + 1) * 128], adst_sb[:],
                             start=True, stop=True)
        ep = sbs.tile([128, Tmax * NH], BF16, tag="ep")
        nc.vector.tensor_tensor(ep[:, 0:T * NH], ae[:, 0:T * NH],
                                gat[:, 0:T, ACOL:ACOL + NH], OP.add)
        lr = sbs.tile([128, Tmax * NH], BF16, tag="lr")
        nc.vector.scalar_tensor_tensor(lr[:, 0:T * NH], ep[:, 0:T * NH],
                                       NEG, ep[:, 0:T * NH], OP.mult, OP.max)
        p = sbs.tile([128, Tmax * NH], F32, tag="p")
        nc.scalar.activation(p[:, 0:T * NH], lr[:, 0:T * NH], AF.Exp)
        p16 = sbs.tile([128, Tmax * NH], BF16, tag="p16")
        nc.scalar.activation(p16[:, 0:T * NH], p[:, 0:T * NH], AF.Copy)

        oacc = psb.tile([128, FEAT], F32, tag="oacc")
        dacc = ae[:, Tmax * NH:Tmax * NH + NH]
        for t in range(T):
            w = sbw.tile([128, FEAT], BF16, tag="w")
            ps = p[:, t * NH:(t + 1) * NH]
            pb = bass.AP(ps.tensor, ps.offset, [ps.ap[0], [1, NH], [0, HID]])
            if lay == 1:
                if t % 6 == 5:
                    for h in range(4):
                        nc.scalar.activation(
                            w[:, h * HID:(h + 1) * HID],
                            gat[:, t, h * HID:(h + 1) * HID],
                            AF.Copy, scale=p[:, t * NH + h:t * NH + h + 1])
                else:
                    nc.vector.tensor_tensor(w[:], gat[:, t, 0:FEAT], pb, OP.mult)
            else:
                if t % 4 == 3:
                    nc.scalar.activation(w[:], gat[:, t, 0:FEAT], AF.Copy,
                                         scale=p[:, t:t + 1])
                else:
                    nc.vector.tensor_tensor(w[:], gat[:, t, 0:FEAT], pb, OP.mult)
            nc.tensor.matmul(oacc[:], S_sb[:, t * 128:(t + 1) * 128], w[:],
                             start=(t == 0), stop=(t == T - 1))
            nc.tensor.matmul(dacc, S_sb[:, t * 128:(t + 1) * 128],
                             p16[:, t * NH:(t + 1) * NH],
                             start=(t == 0), stop=(t == T - 1))
        out_writer(b, base, bs, oacc, dacc)

    st = prefetch(0)
    for b in range(NBLK):
        nxt = prefetch(b + 1) if b + 1 < NBLK else None
        compute(st)
        st = nxt


def _build(meta):
    nc = bacc.Bacc("TRN2", target_bir_lowering=False, debug=False, num_devices=C)
    TT = meta["TT"]

    xT_d = nc.dram_tensor("xT", [F_IN, NS], BF16, kind="ExternalInput").ap()
    rhs1_d = nc.dram_tensor("rhs1", [F_IN, 520], BF16, kind="ExternalInput").ap()
    rhs2_d = nc.dram_tensor("rhs2", [F_IN, 130], BF16, kind="ExternalInput").ap()
    b1r_d = nc.dram_tensor("b1r", [128, 512], F32, kind="ExternalInput").ap()
    b2r_d = nc.dram_tensor("b2r", [128, 128], F32, kind="ExternalInput").ap()
    ident_d = nc.dram_tensor("ident", [128, 128], BF16, kind="ExternalInput").ap()
    idx_d = nc.dram_tensor("idx", [128, TT * 8], I16, kind="ExternalInput").ap()
    S_d = nc.dram_tensor("S", [128, TT * 128], FP8, kind="ExternalInput").ap()
    ST_d = nc.dram_tensor("ST", [128, TT * 128], FP8, kind="ExternalInput").ap()
    out_d = nc.dram_tensor("out", [NS, HID], F32, kind="ExternalOutput").ap()

    meta = dict(meta)
    meta["idx_ap"], meta["S_ap"], meta["ST_ap"] = idx_d, S_d, ST_d

    with tile.TileContext(nc, num_cores=C) as tc:
        with tc.tile_pool(name="dram", bufs=1, space="DRAM") as dram:
            hb1 = dram.tile([NS, ROW1], BF16)
            hfull1 = dram.tile([N, ROW1], BF16, addr_space="Shared")
            ad1 = dram.tile([NS, H1], BF16)
            hb2 = dram.tile([NS, ROW2], BF16)
            hfull2 = dram.tile([N, ROW2], BF16, addr_space="Shared")
            ad2 = dram.tile([NS, 1], BF16)

            # ---------------- phase A: h1 shard + attn scalars ----------------
            with (
                tc.tile_pool(name="a_c", bufs=1) as sbc,
                tc.tile_pool(name="a_w", bufs=3) as sbw,
                tc.tile_pool(name="a_p", bufs=2, space="PSUM") as psp,
            ):
                rhs1_sb = []
                xt_sb = []
                for k in range(4):
                    rt = sbc.tile([128, 520], BF16, name=f"rhs1sb{k}")
                    nc.sync.dma_start(rt[:], rhs1_d[k * 128:(k + 1) * 128, :])
                    rhs1_sb.append(rt)
                    xt = sbc.tile([128, NS], BF16, name=f"xtsb{k}")
                    nc.sync.dma_start(xt[:], xT_d[k * 128:(k + 1) * 128, :])
                    xt_sb.append(xt)
                for b in range(NBLK):
                    bs = CAPS[b]
                    base = b * 128
                    ph = psp.tile([128, 512], F32, tag="ph")
                    pa = psp.tile([128, 8], F32, tag="pa")
                    for k in range(4):
                        nc.tensor.matmul(ph[:bs, :], xt_sb[k][:, base:base + bs],
                                         rhs1_sb[k][:, 0:512],
                                         start=(k == 0), stop=(k == 3))
                        nc.tensor.matmul(pa[:bs, :], xt_sb[k][:, base:base + bs],
                                         rhs1_sb[k][:, 512:520],
                                         start=(k == 0), stop=(k == 3))
                    ha = sbw.tile([128, ROW1], BF16, tag="ha")
                    nc.scalar.activation(ha[:bs, 0:512], ph[:bs, :], AF.Copy)
                    nc.scalar.activation(ha[:bs, 512:516], pa[:bs, 0:4], AF.Copy)
                    nc.vector.memset(ha[:bs, 516:ROW1], 0.0)
                    nc.sync.dma_start(hb1[base:base + bs, :], ha[:bs, :])
                    adsb = sbw.tile([128, H1], BF16, tag="adsb")
                    nc.scalar.activation(adsb[:bs, :], pa[:bs, 4:8], AF.Copy)
                    nc.sync.dma_start(ad1[base:base + bs, :], adsb[:bs, :])

            nc.gpsimd.collective_compute(
                "AllGather", OP.bypass, replica_groups=[list(range(C))],
                ins=[hb1[:].opt()], outs=[hfull1[:].opt()])

            # ---------------- phase B: layer-1 edge pass + h2@W2 ----------------
            with (
                tc.tile_pool(name="b_c", bufs=1) as sbc,
                tc.tile_pool(name="b_m", bufs=2) as sbm,
                tc.tile_pool(name="b_g", bufs=2) as sbg,
                tc.tile_pool(name="b_s", bufs=2) as sbs,
                tc.tile_pool(name="b_w", bufs=4) as sbw,
                tc.tile_pool(name="b_w2", bufs=2) as sbw2,
                tc.tile_pool(name="b_pb", bufs=2, space="PSUM") as psb,
                tc.tile_pool(name="b_pa", bufs=2, space="PSUM") as psa,
                tc.tile_pool(name="b_ph", bufs=1, space="PSUM") as psh,
                tc.tile_pool(name="b_pt", bufs=2, space="PSUM") as pst,
            ):
                b1r_sb = sbc.tile([128, 512], F32, name="b1rsb")
                nc.sync.dma_start(b1r_sb[:], b1r_d)
                ident_sb = sbc.tile([128, 128], BF16, name="identsb")
                nc.sync.dma_start(ident_sb[:], ident_d)
                rhs2_sb = []
                for k in range(4):
                    rt = sbc.tile([128, 130], BF16, name=f"rhs2sb{k}")
                    nc.sync.dma_start(rt[:], rhs2_d[k * 128:(k + 1) * 128, :])
                    rhs2_sb.append(rt)

                def writer_b(b, base, bs, oacc, dacc):
                    den = sbw2.tile([128, 4], F32, tag="den")
                    nc.vector.tensor_scalar_add(den[:], dacc, EPS)
                    rec = sbw2.tile([128, 4], F32, tag="rec")
                    nc.vector.reciprocal(rec[:], den[:])
                    h2b = sbw2.tile([128, 512], BF16, tag="h2b")
                    for h in range(4):
                        nc.vector.scalar_tensor_tensor(
                            h2b[:, h * HID:(h + 1) * HID], oacc[:, h * HID:(h + 1) * HID],
                            rec[:, h:h + 1], b1r_sb[:, h * HID:(h + 1) * HID],
                            OP.mult, OP.add)
                    rl = sbw2.tile([128, 512], BF16, tag="rl")
                    nc.scalar.activation(rl[:], h2b[:], AF.Relu)
                    mn = sbw2.tile([128, 512], BF16, tag="mn")
                    nc.vector.tensor_scalar_min(mn[:], h2b[:], 0.0)
                    em = sbw2.tile([128, 512], BF16, tag="em")
                    nc.scalar.activation(em[:], mn[:], AF.Exp)
                    h2f = sbw2.tile([128, 512], BF16, tag="h2f")
                    nc.vector.scalar_tensor_tensor(h2f[:], em[:], -1.0, rl[:], OP.add, OP.add)
                    hh = psh.tile([128, 130], F32, tag="hh")
                    for k in range(4):
                        tp = pst.tile([128, 128], BF16, tag="tp")
                        nc.tensor.transpose(tp[:], h2f[:, k * 128:(k + 1) * 128], ident_sb[:])
                        h2T = sbw2.tile([128, 128], BF16, tag="h2T")
                        nc.vector.tensor_copy(h2T[:], tp[:])
                        nc.tensor.matmul(hh[:], h2T[:], rhs2_sb[k][:], start=(k == 0), stop=(k == 3))
                    ha2 = sbw2.tile([128, ROW2], BF16, tag="ha2")
                    nc.scalar.activation(ha2[:bs, 0:129], hh[:bs, 0:129], AF.Copy)
                    nc.vector.memset(ha2[:bs, 129:ROW2], 0.0)
                    nc.sync.dma_start(hb2[base:base + bs, :], ha2[:bs, :])
                    adsb2 = sbw2.tile([128, 1], BF16, tag="adsb2")
                    nc.scalar.activation(adsb2[:bs, :], hh[:bs, 129:130], AF.Copy)
                    nc.sync.dma_start(ad2[base:base + bs, :], adsb2[:bs, :])

                _edge_layer(nc, tc, meta, 1, (sbm, sbg, sbs, sbw, psb, psa),
                            ad1, hb1, hfull1, writer_b)

            nc.gpsimd.collective_compute(
                "AllGather", OP.bypass, replica_groups=[list(range(C))],
                ins=[hb2[:].opt()], outs=[hfull2[:].opt()])

            # ---------------- phase D: layer-2 edge pass ----------------
            with (
                tc.tile_pool(name="d_c", bufs=1) as sbc,
                tc.tile_pool(name="d_m", bufs=2) as sbm,
                tc.tile_pool(name="d_g", bufs=2) as sbg,
                tc.tile_pool(name="d_s", bufs=2) as sbs,
                tc.tile_pool(name="d_w", bufs=4) as sbw,
                tc.tile_pool(name="d_w2", bufs=2) as sbw2,
                tc.tile_pool(name="d_pb", bufs=2, space="PSUM") as psb,
                tc.tile_pool(name="d_pa", bufs=2, space="PSUM") as psa,
            ):
                b2r_sb = sbc.tile([128, 128], F32, name="b2rsb")
                nc.sync.dma_start(b2r_sb[:], b2r_d)

                def writer_d(b, base, bs, oacc, dacc):
                    den = sbw2.tile([128, 1], F32, tag="den")
                    nc.vector.tensor_scalar_add(den[:], dacc, EPS)
                    rec = sbw2.tile([128, 1], F32, tag="rec")
                    nc.vector.reciprocal(rec[:], den[:])
                    ofb = sbw2.tile([128, 128], F32, tag="ofb")
                    nc.vector.scalar_tensor_tensor(ofb[:], oacc[:], rec[:, 0:1],
                                                   b2r_sb[:], OP.mult, OP.add)
                    nc.sync.dma_start(out_d[base:base + bs, :], ofb[:bs, :])

                _edge_layer(nc, tc, meta, 2, (sbm, sbg, sbs, sbw, psb, psa),
                            ad2, hb2, hfull2, writer_d)

    nc.compile()
    return nc


# --------------------------------------------------------------------------
# entry point
# --------------------------------------------------------------------------

def kernel(x, edge_index, W1, a_src1, a_dst1, b1, W2, a_src2, a_dst2, b2,
           _trace=False):
    in_maps, meta, perm_order = _prepare(
        x, edge_index, W1, a_src1, a_dst1, b1, W2, a_src2, a_dst2, b2)

    import time as _time
    _t0 = _time.time()
    key = (meta["TT"], tuple(meta["T_lo"]), tuple(meta["T_hi"]),
           tuple(meta["g_lo"]), tuple(meta["g_hi"]))
    if key not in _cache:
        _cache.clear()
        _cache[key] = _build(meta)
    nc = _cache[key]
    print(f"[kernel] build done at {_time.time()-_t0:.1f}s", flush=True)

    kw = {}
    if _trace:
        kw = dict(trace=True)
    res = bass_utils.run_bass_kernel_spmd(nc, in_maps, core_ids=list(range(C)), **kw)

    out = np.empty((N, HID), np.float32)
    for c in range(C):
        out[perm_order[c * NS:(c + 1) * NS]] = res.results[c]["out"]
    kernel._last_result = res
    return out


# revision 13
# speedup vs baseline: 1.1952x; 1.0131x over previous
"""Trainium2 Bass kernel for nn_ClassDiagramGNN: 2-layer GAT on 50k nodes / 850k edges.

Strategy (8 NeuronCores, dst-sharded graph parallel):
  - Host: add self-loops, global LPT of nodes onto cores by degree, per-core
    LPT into 128-wide dst blocks, permute node ids so each block is
    contiguous. Per block, the 128 self-loop edges form a dedicated tile
    whose source rows are a contiguous slice of the local shard (direct DMA,
    no gather). Remaining edges are bucketed by (block, src-half) and packed
    into 128-edge tiles; trailing pad slots get idx=-1 so the gather skips
    them. The one-hot edge->dst scatter matrix S and its transpose ST are
    precomputed per tile in bf16 and streamed in by DMA.
  - Phase A: resident xT; per block h1 = x @ W1 plus attention scalars
    (folded into the matmul), packed as 640-col bf16 rows -> AllGather.
    Per-node adst scalars also go to a compact local table.
  - Edge pass (both layers), software-pipelined per block: prefetch DMAs +
    gathers for block b+1 while computing block b. Attention: ae = ST @ adst
    per tile accumulated into one PSUM strip, then batched add/leaky/exp over
    [128, T*NH]; per-tile w = p * h (split across DVE and ACT), aggregate via
    bf16 S^T @ w matmuls in PSUM, denominator S^T @ p, normalize, ELU,
    h2 @ W2 -> 256-col bf16 rows -> AllGather -> layer-2 pass -> fp32 out.
"""
import sys

for _p in ("/opt/trn_rl_repo",):
    if _p not in sys.path:
        sys.path.append(_p)

import heapq
import numpy as np
import ml_dtypes

import concourse.bass as bass
import concourse.bacc as bacc
import concourse.tile as tile
from concourse import mybir
from concourse import bass_utils

F32 = mybir.dt.float32
BF16 = mybir.dt.bfloat16
FP8 = mybir.dt.float8e4
I16 = mybir.dt.int16
AF = mybir.ActivationFunctionType
OP = mybir.AluOpType
NPBF = ml_dtypes.bfloat16
NPF8 = ml_dtypes.float8_e4m3

# problem constants (hardcoded per contract)
N, F_IN, HID, H1, E = 50000, 512, 128, 4, 800000
NEG = 0.2
C = 8                 # cores
NS = N // C           # 6250 nodes per shard
NBLK = (NS + 127) // 128   # 49 blocks per core
CAPS = [128] * (NBLK - 1) + [NS - 128 * (NBLK - 1)]  # 48x128 + 106
HALF = N // 2         # gather-table half split (int16 index reach)
ROW1 = 640            # layer-1 bf16 row: 512 feat + 4 asrc + pad (1280B)
ROW2 = 256            # layer-2 bf16 row: 128 feat + 1 asrc + pad (512B)
EPS = 1e-16

_cache = {}


def _reconfigure(n, e):
    """Testing hook: shrink the graph (keeps F_IN/HID/H1 fixed)."""
    global N, E, NS, NBLK, CAPS, HALF
    N, E = n, e
    NS = N // C
    NBLK = (NS + 127) // 128
    CAPS = [128] * (NBLK - 1) + [NS - 128 * (NBLK - 1)]
    HALF = N // 2
    _cache.clear()


# --------------------------------------------------------------------------
# host-side preprocessing
# --------------------------------------------------------------------------

def _prepare(x, edge_index, W1, a_src1, a_dst1, b1, W2, a_src2, a_dst2, b2):
    # self-loops handled as dedicated per-block tiles; bucket only real edges
    src = edge_index[0].astype(np.int64)
    dst = edge_index[1].astype(np.int64)
    deg = np.bincount(dst, minlength=N) + 1  # +1 self loop

    # global LPT of nodes onto cores by degree (balances edge counts), then
    # per-core LPT into blocks
    perm_pos = np.empty(N, dtype=np.int64)    # orig id -> permuted global pos
    perm_order = np.empty(N, dtype=np.int64)  # permuted pos -> orig id
    order_g = np.argsort(-deg, kind="stable")
    cheap = [(0, 0, ci) for ci in range(C)]
    heapq.heapify(cheap)
    core_nodes = [[] for _ in range(C)]
    for nid in order_g:
        while True:
            load, used, ci = heapq.heappop(cheap)
            if used < NS:
                break
        core_nodes[ci].append(nid)
        heapq.heappush(cheap, (load + int(deg[nid]), used + 1, ci))
    for c in range(C):
        ids = np.array(core_nodes[c])
        d = deg[ids]
        order = np.argsort(-d, kind="stable")
        heap = [(0, 0, i) for i in range(NBLK)]
        heapq.heapify(heap)
        assign = [[] for _ in range(NBLK)]
        for li in order:
            while True:
                load, used, bi = heapq.heappop(heap)
                if used < CAPS[bi]:
                    break
            assign[bi].append(li)
            heapq.heappush(heap, (load + int(d[li]), used + 1, bi))
        loads = [(-(sum(int(d[li]) for li in assign[bi])), bi) for bi in range(NBLK)]
        # full blocks first (the short last block must keep its position so
        # CAPS stays aligned), sorted by descending load
        fulls = sorted([x for x in loads if len(assign[x[1]]) == 128])
        rest = [x for x in loads if len(assign[x[1]]) != 128]
        border = [bi for _, bi in fulls] + [bi for _, bi in rest]
        pos = 0
        for bi in border:
            for li in assign[bi]:
                g = c * NS + pos
                perm_pos[ids[li]] = g
                perm_order[g] = ids[li]
                pos += 1

    src_p = perm_pos[src]
    dst_p = perm_pos[dst]
    core = dst_p // NS
    blk = (dst_p % NS) // 128
    halfv = (src_p >= HALF).astype(np.int64)
    key = (core * NBLK + blk) * 2 + halfv
    eorder = np.argsort(key, kind="stable")
    counts = np.bincount(key, minlength=C * NBLK * 2).reshape(C, NBLK, 2)

    # cross-core uniform tile counts per (block, half); +1 self tile per block
    T_lo = -(-counts[:, :, 0].max(axis=0) // 128)  # ceil
    T_hi = -(-counts[:, :, 1].max(axis=0) // 128)
    T_all = 1 + T_lo + T_hi                        # self tile first
    TT = int(T_all.sum())
    Tmax = int(T_all.max())
    toff = np.zeros(NBLK, np.int64)
    toff[1:] = np.cumsum(T_all)[:-1]

    src_sorted = src_p[eorder]
    dloc_sorted = (dst_p[eorder] % NS) % 128
    starts = np.zeros(C * NBLK * 2 + 1, np.int64)
    starts[1:] = np.cumsum(counts.reshape(-1))

    idx_all = np.zeros((C, TT * 128), np.int16)           # pad -> row 0
    dc_all = np.full((C, TT * 128), 999, np.int64)        # pad -> no dst
    nreal = np.zeros((C, NBLK, 2), np.int64)
    for c in range(C):
        for b in range(NBLK):
            # self tile: dst-local row i <- node base+i (contiguous source)
            bs = CAPS[b]
            slot0 = toff[b] * 128
            dc_all[c, slot0:slot0 + bs] = np.arange(bs)
            for h in range(2):
                k = (c * NBLK + b) * 2 + h
                s0, s1 = starts[k], starts[k + 1]
                n = s1 - s0
                nreal[c, b, h] = n
                if n == 0:
                    continue
                slot0 = (toff[b] + 1 + (T_lo[b] if h else 0)) * 128
                seg = src_sorted[s0:s1]
                if h:
                    seg = seg - HALF
                idx_all[c, slot0:slot0 + n] = seg.astype(np.int16)
                dc_all[c, slot0:slot0 + n] = dloc_sorted[s0:s1]

    # full-range gathers (negative-index skipping crashes the gather ucode,
    # so pad slots gather row 0 instead)
    g_lo = T_lo * 128
    g_hi = T_hi * 128

    # weights: fold per-head attention projections into the linear transforms
    W1_64 = np.asarray(W1, np.float64)
    Dsrc1 = np.zeros((H1 * HID, H1), np.float64)
    Ddst1 = np.zeros((H1 * HID, H1), np.float64)
    a_src1_64 = np.asarray(a_src1, np.float64)
    a_dst1_64 = np.asarray(a_dst1, np.float64)
    for h in range(H1):
        Dsrc1[h * HID:(h + 1) * HID, h] = a_src1_64[h]
        Ddst1[h * HID:(h + 1) * HID, h] = a_dst1_64[h]
    rhs1 = np.concatenate(
        [np.asarray(W1, np.float32),
         (W1_64 @ Dsrc1).astype(np.float32),
         (W1_64 @ Ddst1).astype(np.float32)], axis=1).astype(NPBF)  # [512, 520]
    W2_64 = np.asarray(W2, np.float64)
    rhs2 = np.concatenate(
        [np.asarray(W2, np.float32),
         (W2_64 @ np.asarray(a_src2, np.float64)[0][:, None]).astype(np.float32),
         (W2_64 @ np.asarray(a_dst2, np.float64)[0][:, None]).astype(np.float32)],
        axis=1).astype(NPBF)                                        # [512, 130]

    ident = np.eye(128, dtype=NPBF)
    b1r = np.tile(np.asarray(b1, np.float32)[None, :], (128, 1))
    b2r = np.tile(np.asarray(b2, np.float32)[None, :], (128, 1))

    slots = np.arange(TT * 128)
    tloc = slots // 128
    posi = slots % 128

    xnp = np.asarray(x, np.float32)
    in_maps = []
    for c in range(C):
        rows = perm_order[c * NS:(c + 1) * NS]
        xT = np.ascontiguousarray(xnp[rows].T).astype(NPBF)     # [512, 6250]
        idx_w = np.tile(idx_all[c].reshape(-1, 16).T, (8, 1))   # [128, TT*8]
        dcv = dc_all[c]
        valid = dcv < 128
        dv = dcv[valid]
        tv = tloc[valid]
        pv = posi[valid]
        S_all = np.zeros((128, TT * 128), NPF8)
        S_all[pv, tv * 128 + dv] = 1
        ST_all = np.zeros((128, TT * 128), NPF8)
        ST_all[dv, tv * 128 + pv] = 1
        in_maps.append({
            "xT": xT, "rhs1": rhs1, "rhs2": rhs2,
            "b1r": b1r, "b2r": b2r, "ident": ident,
            "idx": np.ascontiguousarray(idx_w),
            "S": S_all, "ST": ST_all,
        })

    meta = {
        "T_lo": [int(v) for v in T_lo],
        "T_hi": [int(v) for v in T_hi],
        "toff": [int(v) for v in toff],
        "g_lo": [int(v) for v in g_lo],
        "g_hi": [int(v) for v in g_hi],
        "TT": TT,
        "Tmax": Tmax,
    }
    return in_maps, meta, perm_order


# --------------------------------------------------------------------------
# device program
# --------------------------------------------------------------------------

def _edge_layer(nc, tc, meta, lay, pools, ad_my, hb_my, hfull, out_writer):
    """Software-pipelined per-block edge pass shared by both GAT layers.

    lay=1: ROW=640, 4 heads, feat cols 0:512, asrc 512:516
    lay=2: ROW=256, 1 head, feat cols 0:128, asrc 128:129
    """
    sbm, sbg, sbs, sbw, psb, psa = pools
    ROW = ROW1 if lay == 1 else ROW2
    NH = H1 if lay == 1 else 1
    FEAT = NH * HID
    ACOL = FEAT
    idx_d, S_d, ST_d = meta["idx_ap"], meta["S_ap"], meta["ST_ap"]
    Tmax = meta["Tmax"]

    # one-time zero of the gather-pool buffers so skipped pad slots never
    # expose uninitialized SBUF (NaN/Inf) to the attention math
    for _ in range(sbg.bufs):
        gz = sbg.tile([128, Tmax, ROW], BF16, tag="gat")
        nc.vector.memset(gz[:, :, :], 0.0)

    def prefetch(b):
        bs = CAPS[b]
        base = b * 128
        T_lo, T_hi = meta["T_lo"][b], meta["T_hi"][b]
        T = 1 + T_lo + T_hi
        boff = meta["toff"][b]
        g_lo, g_hi = meta["g_lo"][b], meta["g_hi"][b]

        idx_sb = sbm.tile([128, Tmax * 8], I16, tag="idx")
        nc.sync.dma_start(idx_sb[:, 0:T * 8], idx_d[:, boff * 8:(boff + T) * 8])
        S_sb = sbm.tile([128, Tmax * 128], FP8, tag="S")
        nc.sync.dma_start(S_sb[:, 0:T * 128], S_d[:, boff * 128:(boff + T) * 128])
        ST_sb = sbm.tile([128, Tmax * 128], FP8, tag="ST")
        nc.sync.dma_start(ST_sb[:, 0:T * 128], ST_d[:, boff * 128:(boff + T) * 128])
        adst_sb = sbm.tile([128, NH], BF16, tag="adst")
        if bs < 128:
            nc.vector.memset(adst_sb[:], 0.0)
        nc.sync.dma_start(adst_sb[:bs], ad_my[base:base + bs, :])

        gat = sbg.tile([128, Tmax, ROW], BF16, tag="gat")
        # self tile: contiguous local rows
        nc.sync.dma_start(gat[:bs, 0, :], hb_my[base:base + bs, :])
        if g_lo:
            nc.gpsimd.dma_gather(
                gat[:, 1:1 + g_lo // 128, :], hfull[0:HALF, :],
                idx_sb[:, 8:(1 + g_lo // 128) * 8],
                g_lo, g_lo, ROW, elem_step=ROW, single_packet=False)
        if g_hi:
            t0 = 1 + T_lo
            nc.gpsimd.dma_gather(
                gat[:, t0:t0 + g_hi // 128, :], hfull[HALF:N, :],
                idx_sb[:, t0 * 8:(t0 + g_hi // 128) * 8],
                g_hi, g_hi, ROW, elem_step=ROW, single_packet=False)
        return (b, bs, base, T, idx_sb, S_sb, ST_sb, adst_sb, gat)

    def compute(st):
        b, bs, base, T, idx_sb, S_sb, ST_sb, adst_sb, gat = st
        ae = psa.tile([128, Tmax * NH + NH], F32, tag="ae")
        for t in range(T):
            nc.tensor.matmul(ae[:, t * NH:(t + 1) * NH],
                             ST_sb[:, t * 128:(t + 1) * 128], adst_sb[:],
                             start=True, stop=True)
        ep = sbs.tile([128, Tmax * NH], BF16, tag="ep")
        nc.vector.tensor_tensor(ep[:, 0:T * NH], ae[:, 0:T * NH],
                                gat[:, 0:T, ACOL:ACOL + NH], OP.add)
        lr = sbs.tile([128, Tmax * NH], BF16, tag="lr")
        nc.vector.scalar_tensor_tensor(lr[:, 0:T * NH], ep[:, 0:T * NH],
                                       NEG, ep[:, 0:T * NH], OP.mult, OP.max)
        p = sbs.tile([128, Tmax * NH], F32, tag="p")
        nc.scalar.activation(p[:, 0:T * NH], lr[:, 0:T * NH], AF.Exp)
        p16 = sbs.tile([128, Tmax * NH], BF16, tag="p16")
        nc.scalar.activation(p16[:, 0:T * NH], p[:, 0:T * NH], AF.Copy)

        oacc = psb.tile([128, FEAT], F32, tag="oacc")
        dacc = ae[:, Tmax * NH:Tmax * NH + NH]
        for t in range(T):
            w = sbw.tile([128, FEAT], BF16, tag="w")
            ps = p[:, t * NH:(t + 1) * NH]
            pb = bass.AP(ps.tensor, ps.offset, [ps.ap[0], [1, NH], [0, HID]])
            if lay == 1:
                if t % 6 == 5:
                    for h in range(4):
                        nc.scalar.activation(
                            w[:, h * HID:(h + 1) * HID],
                            gat[:, t, h * HID:(h + 1) * HID],
                            AF.Copy, scale=p[:, t * NH + h:t * NH + h + 1])
                else:
                    nc.vector.tensor_tensor(w[:], gat[:, t, 0:FEAT], pb, OP.mult)
            else:
                if t % 4 == 3:
                    nc.scalar.activation(w[:], gat[:, t, 0:FEAT], AF.Copy,
                                         scale=p[:, t:t + 1])
                else:
                    nc.vector.tensor_tensor(w[:], gat[:, t, 0:FEAT], pb, OP.mult)
            nc.tensor.matmul(oacc[:], S_sb[:, t * 128:(t + 1) * 128], w[:],
                             start=(t == 0), stop=(t == T - 1))
            nc.tensor.matmul(dacc, S_sb[:, t * 128:(t + 1) * 128],
                             p16[:, t * NH:(t + 1) * NH],
                             start=(t == 0), stop=(t == T - 1))
        out_writer(b, base, bs, oacc, dacc)

    pending = [prefetch(0)]
    if NBLK > 1:
        pending.append(prefetch(1))
    for b in range(NBLK):
        if b + 2 < NBLK:
            pending.append(prefetch(b + 2))
        compute(pending.pop(0))


def _build(meta):
    nc = bacc.Bacc("TRN2", target_bir_lowering=False, debug=False, num_devices=C)
    TT = meta["TT"]

    xT_d = nc.dram_tensor("xT", [F_IN, NS], BF16, kind="ExternalInput").ap()
    rhs1_d = nc.dram_tensor("rhs1", [F_IN, 520], BF16, kind="ExternalInput").ap()
    rhs2_d = nc.dram_tensor("rhs2", [F_IN, 130], BF16, kind="ExternalInput").ap()
    b1r_d = nc.dram_tensor("b1r", [128, 512], F32, kind="ExternalInput").ap()
    b2r_d = nc.dram_tensor("b2r", [128, 128], F32, kind="ExternalInput").ap()
    ident_d = nc.dram_tensor("ident", [128, 128], BF16, kind="ExternalInput").ap()
    idx_d = nc.dram_tensor("idx", [128, TT * 8], I16, kind="ExternalInput").ap()
    S_d = nc.dram_tensor("S", [128, TT * 128], FP8, kind="ExternalInput").ap()
    ST_d = nc.dram_tensor("ST", [128, TT * 128], FP8, kind="ExternalInput").ap()
    out_d = nc.dram_tensor("out", [NS, HID], F32, kind="ExternalOutput").ap()

    meta = dict(meta)
    meta["idx_ap"], meta["S_ap"], meta["ST_ap"] = idx_d, S_d, ST_d

    with tile.TileContext(nc, num_cores=C) as tc:
        with tc.tile_pool(name="dram", bufs=1, space="DRAM") as dram:
            hb1 = dram.tile([NS, ROW1], BF16)
            hfull1 = dram.tile([N, ROW1], BF16, addr_space="Shared")
            ad1 = dram.tile([NS, H1], BF16)
            hb2 = dram.tile([NS, ROW2], BF16)
            hfull2 = dram.tile([N, ROW2], BF16, addr_space="Shared")
            ad2 = dram.tile([NS, 1], BF16)

            # ---------------- phase A: h1 shard + attn scalars ----------------
            with (
                tc.tile_pool(name="a_c", bufs=1) as sbc,
                tc.tile_pool(name="a_w", bufs=3) as sbw,
                tc.tile_pool(name="a_p", bufs=2, space="PSUM") as psp,
            ):
                rhs1_sb = []
                xt_sb = []
                for k in range(4):
                    rt = sbc.tile([128, 520], BF16, name=f"rhs1sb{k}")
                    nc.sync.dma_start(rt[:], rhs1_d[k * 128:(k + 1) * 128, :])
                    rhs1_sb.append(rt)
                    xt = sbc.tile([128, NS], BF16, name=f"xtsb{k}")
                    nc.sync.dma_start(xt[:], xT_d[k * 128:(k + 1) * 128, :])
                    xt_sb.append(xt)
                for b in range(NBLK):
                    bs = CAPS[b]
                    base = b * 128
                    ph = psp.tile([128, 512], F32, tag="ph")
                    pa = psp.tile([128, 8], F32, tag="pa")
                    for k in range(4):
                        nc.tensor.matmul(ph[:bs, :], xt_sb[k][:, base:base + bs],
                                         rhs1_sb[k][:, 0:512],
                                         start=(k == 0), stop=(k == 3))
                        nc.tensor.matmul(pa[:bs, :], xt_sb[k][:, base:base + bs],
                                         rhs1_sb[k][:, 512:520],
                                         start=(k == 0), stop=(k == 3))
                    ha = sbw.tile([128, ROW1], BF16, tag="ha")
                    nc.scalar.activation(ha[:bs, 0:512], ph[:bs, :], AF.Copy)
                    nc.scalar.activation(ha[:bs, 512:516], pa[:bs, 0:4], AF.Copy)
                    nc.vector.memset(ha[:bs, 516:ROW1], 0.0)
                    nc.sync.dma_start(hb1[base:base + bs, :], ha[:bs, :])
                    adsb = sbw.tile([128, H1], BF16, tag="adsb")
                    nc.scalar.activation(adsb[:bs, :], pa[:bs, 4:8], AF.Copy)
                    nc.sync.dma_start(ad1[base:base + bs, :], adsb[:bs, :])

            nc.gpsimd.collective_compute(
                "AllGather", OP.bypass, replica_groups=[list(range(C))],
                ins=[hb1[:].opt()], outs=[hfull1[:].opt()])

            # ---------------- phase B: layer-1 edge pass + h2@W2 ----------------
            with (
                tc.tile_pool(name="b_c", bufs=1) as sbc,
                tc.tile_pool(name="b_m", bufs=3) as sbm,
                tc.tile_pool(name="b_g", bufs=3) as sbg,
                tc.tile_pool(name="b_s", bufs=2) as sbs,
                tc.tile_pool(name="b_w", bufs=4) as sbw,
                tc.tile_pool(name="b_w2", bufs=2) as sbw2,
                tc.tile_pool(name="b_pb", bufs=2, space="PSUM") as psb,
                tc.tile_pool(name="b_pa", bufs=2, space="PSUM") as psa,
                tc.tile_pool(name="b_ph", bufs=1, space="PSUM") as psh,
                tc.tile_pool(name="b_pt", bufs=2, space="PSUM") as pst,
            ):
                b1r_sb = sbc.tile([128, 512], F32, name="b1rsb")
                nc.sync.dma_start(b1r_sb[:], b1r_d)
                ident_sb = sbc.tile([128, 128], BF16, name="identsb")
                nc.sync.dma_start(ident_sb[:], ident_d)
                rhs2_sb = []
                for k in range(4):
                    rt = sbc.tile([128, 130], BF16, name=f"rhs2sb{k}")
                    nc.sync.dma_start(rt[:], rhs2_d[k * 128:(k + 1) * 128, :])
                    rhs2_sb.append(rt)

                def writer_b(b, base, bs, oacc, dacc):
                    den = sbw2.tile([128, 4], F32, tag="den")
                    nc.vector.tensor_scalar_add(den[:], dacc, EPS)
                    rec = sbw2.tile([128, 4], F32, tag="rec")
                    nc.vector.reciprocal(rec[:], den[:])
                    h2b = sbw2.tile([128, 512], BF16, tag="h2b")
                    for h in range(4):
                        nc.vector.scalar_tensor_tensor(
                            h2b[:, h * HID:(h + 1) * HID], oacc[:, h * HID:(h + 1) * HID],
                            rec[:, h:h + 1], b1r_sb[:, h * HID:(h + 1) * HID],
                            OP.mult, OP.add)
                    rl = sbw2.tile([128, 512], BF16, tag="rl")
                    nc.scalar.activation(rl[:], h2b[:], AF.Relu)
                    mn = sbw2.tile([128, 512], BF16, tag="mn")
                    nc.vector.tensor_scalar_min(mn[:], h2b[:], 0.0)
                    em = sbw2.tile([128, 512], BF16, tag="em")
                    nc.scalar.activation(em[:], mn[:], AF.Exp)
                    h2f = sbw2.tile([128, 512], BF16, tag="h2f")
                    nc.vector.scalar_tensor_tensor(h2f[:], em[:], -1.0, rl[:], OP.add, OP.add)
                    hh = psh.tile([128, 130], F32, tag="hh")
                    for k in range(4):
                        tp = pst.tile([128, 128], BF16, tag="tp")
                        nc.tensor.transpose(tp[:], h2f[:, k * 128:(k + 1) * 128], ident_sb[:])
                        h2T = sbw2.tile([128, 128], BF16, tag="h2T")
                        nc.vector.tensor_copy(h2T[:], tp[:])
                        nc.tensor.matmul(hh[:], h2T[:], rhs2_sb[k][:], start=(k == 0), stop=(k == 3))
                    ha2 = sbw2.tile([128, ROW2], BF16, tag="ha2")
                    nc.scalar.activation(ha2[:bs, 0:129], hh[:bs, 0:129], AF.Copy)
                    nc.vector.memset(ha2[:bs, 129:ROW2], 0.0)
                    nc.sync.dma_start(hb2[base:base + bs, :], ha2[:bs, :])
                    adsb2 = sbw2.tile([128, 1], BF16, tag="adsb2")
                    nc.scalar.activation(adsb2[:bs, :], hh[:bs, 129:130], AF.Copy)
                    nc.sync.dma_start(ad2[base:base + bs, :], adsb2[:bs, :])

                _edge_layer(nc, tc, meta, 1, (sbm, sbg, sbs, sbw, psb, psa),
                            ad1, hb1, hfull1, writer_b)

            nc.gpsimd.collective_compute(
                "AllGather", OP.bypass, replica_groups=[list(range(C))],
                ins=[hb2[:].opt()], outs=[hfull2[:].opt()])

            # ---------------- phase D: layer-2 edge pass ----------------
            with (
                tc.tile_pool(name="d_c", bufs=1) as sbc,
                tc.tile_pool(name="d_m", bufs=3) as sbm,
                tc.tile_pool(name="d_g", bufs=3) as sbg,
                tc.tile_pool(name="d_s", bufs=2) as sbs,
                tc.tile_pool(name="d_w", bufs=4) as sbw,
                tc.tile_pool(name="d_w2", bufs=2) as sbw2,
                tc.tile_pool(name="d_pb", bufs=2, space="PSUM") as psb,
                tc.tile_pool(name="d_pa", bufs=2, space="PSUM") as psa,
            ):
                b2r_sb = sbc.tile([128, 128], F32, name="b2rsb")
                nc.sync.dma_start(b2r_sb[:], b2r_d)

                def writer_d(b, base, bs, oacc, dacc):
                    den = sbw2.tile([128, 1], F32, tag="den")
                    nc.vector.tensor_scalar_add(den[:], dacc, EPS)
                    rec = sbw2.tile([128, 1], F32, tag="rec")
                    nc.vector.reciprocal(rec[:], den[:])
                    ofb = sbw2.tile([128, 128], F32, tag="ofb")
                    nc.vector.scalar_tensor_tensor(ofb[:], oacc[:], rec[:, 0:1],
                                                   b2r_sb[:], OP.mult, OP.add)
                    nc.sync.dma_start(out_d[base:base + bs, :], ofb[:bs, :])

                _edge_layer(nc, tc, meta, 2, (sbm, sbg, sbs, sbw, psb, psa),
                            ad2, hb2, hfull2, writer_d)

    nc.compile()
    return nc


# --------------------------------------------------------------------------
# entry point
# --------------------------------------------------------------------------

def kernel(x, edge_index, W1, a_src1, a_dst1, b1, W2, a_src2, a_dst2, b2,
           _trace=False):
    in_maps, meta, perm_order = _prepare(
        x, edge_index, W1, a_src1, a_dst1, b1, W2, a_src2, a_dst2, b2)

    import time as _time
    _t0 = _time.time()
    key = (meta["TT"], tuple(meta["T_lo"]), tuple(meta["T_hi"]),
           tuple(meta["g_lo"]), tuple(meta["g_hi"]))
    if key not in _cache:
        _cache.clear()
        _cache[key] = _build(meta)
    nc = _cache[key]
    print(f"[kernel] build done at {_time.time()-_t0:.1f}s", flush=True)

    kw = {}
    if _trace:
        kw = dict(trace=True)
    res = bass_utils.run_bass_kernel_spmd(nc, in_maps, core_ids=list(range(C)), **kw)

    out = np.empty((N, HID), np.float32)
    for c in range(C):
        out[perm_order[c * NS:(c + 1) * NS]] = res.results[c]["out"]
    kernel._last_result = res
    return out
